# revision 8
# baseline (speedup 1.0000x reference)
"""AgentAttention Trainium2 kernel (8-core SPMD).

Sharding: data-parallel over B (4) x head-group parallel (2 groups of 8
heads).  Core i handles batch i//2, head-group i%2 (512 of 1024 channels).

Per-core pipeline (all matmuls bf16 with fp32 PSUM accumulation):
  A) stream L-tiles: cast-load q/v, DMA-transpose to q^T/v^T, compute
     v_proj = v @ W_v + b_v (lepe), pool q -> agent tokens.
  B) stream k: transpose to k^T, stage-1 scores s1 = (ag*scale) @ k^T with
     an_bias folded in via an identity matmul, exp (softmax w/o max --
     scores are provably small for this problem's distributions).
  C) per head-pair: transpose exp(s1), agent_v = attn1 @ v_proj (with the
     softmax denominator from the ACT accumulator), normalize.
  D) stage-2 scores s2^T = (ag*scale) @ q^T + na_bias, exp.
  E) x = attn2 @ agent_v via block-diagonal matmuls (+ ones-columns for the
     stage-2 softmax denominator), normalize, add lepe, store.
"""

import sys

for _p in ("/opt/trn_rl_repo",):
    if _p not in sys.path:
        sys.path.insert(0, _p)

import numpy as np
import ml_dtypes

B, L, C, H, A = 4, 4096, 1024, 16, 49
CH = 512          # channels per head-group (per core)
HD = 64           # head dim
PAIRS = 4         # head pairs per core (2 heads = 128 ch each)
NP = 128          # padded pair rows: h0 at [0:A], h1 at [64:64+A]
NL = L // 128     # 32 L-tiles
NCK = L // 512    # 8 chunks of 512 along L
SCALE = HD ** -0.5

_CACHE = {}


def _pool_matrix_T():
    """P^T (L, A) for AdaptiveAvgPool1d, fp32."""
    Pt = np.zeros((L, A), dtype=np.float32)
    for i in range(A):
        s = (i * L) // A
        e = -((-(i + 1) * L) // A)
        Pt[s:e, i] = 1.0 / (e - s)
    return Pt


def _build_program():
    import concourse.bass as bass
    import concourse.mybir as mybir
    import concourse.tile as tile
    from concourse import bacc
    from concourse.bass import ts, ds

    dt = mybir.dt
    F32, BF16 = dt.float32, dt.bfloat16
    Exp = mybir.ActivationFunctionType.Exp
    X = mybir.AxisListType.X

    nc = bacc.Bacc("TRN2", target_bir_lowering=False, debug=False,
                   enable_asserts=False, num_devices=8)

    q_d = nc.dram_tensor("q_sh", [L, CH], F32, kind="ExternalInput").ap()
    k_d = nc.dram_tensor("k_sh", [L, CH], F32, kind="ExternalInput").ap()
    v_d = nc.dram_tensor("v_sh", [L, C], F32, kind="ExternalInput").ap()
    w_d = nc.dram_tensor("w_sh", [C, CH], F32, kind="ExternalInput").ap()
    bv_d = nc.dram_tensor("bv_sh", [1, CH], F32, kind="ExternalInput").ap()
    anb_d = nc.dram_tensor("anb_sh", [8, A, L], F32, kind="ExternalInput").ap()
    nab_d = nc.dram_tensor("nab_sh", [8, A, L], F32, kind="ExternalInput").ap()
    pt_d = nc.dram_tensor("pt_sh", [L, A], BF16, kind="ExternalInput").ap()
    iexta_d = nc.dram_tensor("iexta_sh", [A, NP], BF16, kind="ExternalInput").ap()
    iextb_d = nc.dram_tensor("iextb_sh", [A, NP], BF16, kind="ExternalInput").ap()
    ones2_d = nc.dram_tensor("ones2_sh", [NP, 2], BF16, kind="ExternalInput").ap()
    id_d = nc.dram_tensor("id_sh", [A, A], BF16, kind="ExternalInput").ap()
    out_d = nc.dram_tensor("out_sh", [L, CH], F32, kind="ExternalOutput").ap()

    from contextlib import ExitStack
    with tile.TileContext(nc) as tc:
        with ExitStack() as _es:
            def _pool(**kw):
                return _es.enter_context(tc.tile_pool(**kw))
            consts = _pool(name="consts", bufs=1)
            qT_pool = _pool(name="qT", bufs=1)
            vp32_pool = _pool(name="vp32", bufs=NL)
            vpbf_pool = _pool(name="vpbf", bufs=NL)
            bd_pool = _pool(name="bd", bufs=PAIRS)
            avbd_pool = _pool(name="avbd", bufs=PAIRS)
            agsb_pool = _pool(name="agsb", bufs=1)
            srp = _pool(name="srp", bufs=2)
            acc_pool = _pool(name="acc1", bufs=PAIRS)
            qn_pool = _pool(name="qn", bufs=2)
            vn_pool = _pool(name="vn", bufs=2)
            vt_pool = _pool(name="vt", bufs=2)
            kn_pool = _pool(name="kn", bufs=2)
            kt_pool = _pool(name="kt", bufs=2)
            bias_pool = _pool(name="bias", bufs=2)
            e12_pool = _pool(name="e12", bufs=PAIRS)
            e1t_pool = _pool(name="e1t", bufs=1)
            r2_pool = _pool(name="r2p", bufs=2)
            ps_big = _pool(name="ps_big", bufs=2, space="PSUM")
            ps_score = _pool(name="ps_score", bufs=2, space="PSUM")
            ps_agent = _pool(name="ps_agent", bufs=1, space="PSUM")
            ps_small = _pool(name="ps_small", bufs=2, space="PSUM")

            # ---- constants ----
            w_bf = consts.tile([128, C // 128, CH], BF16, tag="w")
            nc.gpsimd.dma_start(out=w_bf, in_=w_d.rearrange("(c p) n -> p c n", p=128))
            pt = consts.tile([128, NL, A], BF16, tag="pt")
            nc.sync.dma_start(out=pt, in_=pt_d.rearrange("(n p) a -> p n a", p=128))
            iexta = consts.tile([A, NP], BF16, tag="iexta")
            nc.sync.dma_start(out=iexta, in_=iexta_d)
            iextb = consts.tile([A, NP], BF16, tag="iextb")
            nc.sync.dma_start(out=iextb, in_=iextb_d)
            ones2 = consts.tile([NP, 2], BF16, tag="ones2")
            nc.sync.dma_start(out=ones2, in_=ones2_d)
            id49 = consts.tile([A, A], BF16, tag="id49")
            nc.sync.dma_start(out=id49, in_=id_d)
            bv_bf = consts.tile([1, CH], BF16, tag="bv")
            nc.gpsimd.dma_start(out=bv_bf, in_=bv_d)
            onesrow = consts.tile([1, 128], BF16, tag="onesrow")
            nc.vector.memset(onesrow, 1.0)

            qT = qT_pool.tile([128, PAIRS, L], BF16, tag="qT")
            vp32s, vpbfs, bds, avbds, exp2s = [], [], [], [], []

            # ================= PHASE A: q/v stream =================
            agent_ps = ps_agent.tile([A, CH], F32, tag="agent")
            for l0 in range(NL):
                qn = qn_pool.tile([128, CH], BF16, tag="qn")
                nc.gpsimd.dma_start(out=qn, in_=q_d[ts(l0, 128), :])
                for p in range(PAIRS):
                    nc.sync.dma_start(out=qT[:, p, ts(l0, 128)],
                                      in_=qn[:, ts(p, 128)], transpose=True)
                nc.tensor.matmul(agent_ps, lhsT=pt[:, l0, :], rhs=qn,
                                 start=(l0 == 0), stop=(l0 == NL - 1))

                vn = vn_pool.tile([128, C], BF16, tag="vn")
                nc.gpsimd.dma_start(out=vn, in_=v_d[ts(l0, 128), :])
                vt = vt_pool.tile([128, C // 128, 128], BF16, tag="vt")
                for cb in range(C // 128):
                    nc.sync.dma_start(out=vt[:, cb, :],
                                      in_=vn[:, ts(cb, 128)], transpose=True)
                vps = ps_big.tile([128, CH], F32, tag="big")
                for cb in range(C // 128):
                    nc.tensor.matmul(vps, lhsT=vt[:, cb, :], rhs=w_bf[:, cb, :],
                                     start=(cb == 0), stop=False)
                nc.tensor.matmul(vps, lhsT=onesrow, rhs=bv_bf,
                                 start=False, stop=True)
                vp32 = vp32_pool.tile([128, CH], F32, tag="vp32")
                nc.vector.tensor_copy(out=vp32, in_=vps)
                vpbf = vpbf_pool.tile([128, CH], BF16, tag="vpbf")
                nc.scalar.copy(out=vpbf, in_=vps)
                vp32s.append(vp32)
                vpbfs.append(vpbf)

            # agent -> scaled, transposed, block-diagonal lhsT
            agsb = agsb_pool.tile([A, CH], BF16, tag="agsb")
            nc.vector.tensor_scalar_mul(out=agsb, in0=agent_ps, scalar1=SCALE)
            for p in range(PAIRS):
                agt_ps = ps_small.tile([128, A], BF16, tag="small")
                nc.tensor.transpose(agt_ps, agsb[:, ts(p, 128)], id49)
                bd = bd_pool.tile([128, NP], BF16, tag="bd")
                nc.vector.memset(bd, 0.0)
                nc.vector.tensor_copy(out=bd[0:64, 0:A], in_=agt_ps[0:64, :])
                nc.vector.tensor_copy(out=bd[64:128, 64:64 + A], in_=agt_ps[64:128, :])
                bds.append(bd)

            # ============ PHASE B: k stream + stage-1 scores ============
            exp1s = [e12_pool.tile([NP, L], BF16, tag="e12", name=f"exp1_{i}")
                     for i in range(PAIRS)]
            accs = [acc_pool.tile([NP, NCK], F32, tag="acc", name=f"acc1_{i}")
                    for i in range(PAIRS)]
            for c in range(NCK):
                ktc = kt_pool.tile([128, PAIRS, 512], BF16, tag="kt")
                for j in range(4):
                    l0 = 4 * c + j
                    kn = kn_pool.tile([128, CH], BF16, tag="kn")
                    nc.gpsimd.dma_start(out=kn, in_=k_d[ts(l0, 128), :])
                    for p in range(PAIRS):
                        nc.sync.dma_start(out=ktc[:, p, ts(j, 128)],
                                          in_=kn[:, ts(p, 128)], transpose=True)
                for p in range(PAIRS):
                    anbt0 = bias_pool.tile([A, 512], BF16, tag="bias")
                    nc.gpsimd.dma_start(out=anbt0,
                                        in_=anb_d[2 * p, :, ts(c, 512)])
                    anbt1 = bias_pool.tile([A, 512], BF16, tag="bias")
                    nc.gpsimd.dma_start(out=anbt1,
                                        in_=anb_d[2 * p + 1, :, ts(c, 512)])
                    s1ps = ps_score.tile([NP, 512], F32, tag="score")
                    nc.tensor.matmul(s1ps, lhsT=bds[p], rhs=ktc[:, p, :],
                                     start=True, stop=False)
                    nc.tensor.matmul(s1ps, lhsT=iexta, rhs=anbt0,
                                     start=False, stop=False)
                    nc.tensor.matmul(s1ps, lhsT=iextb, rhs=anbt1,
                                     start=False, stop=True)
                    nc.scalar.activation(out=exp1s[p][:, ts(c, 512)], in_=s1ps,
                                         func=Exp,
                                         accum_out=accs[p][:, c:c + 1])

            # ============ PHASE C: agent_v per pair ============
            for p in range(PAIRS):
                e1t = e1t_pool.tile([128, NL, NP], BF16, tag="e1t")
                for l0 in range(NL):
                    nc.sync.dma_start(out=e1t[:, l0, :],
                                      in_=exp1s[p][:, ts(l0, 128)],
                                      transpose=True)
                s1sum = srp.tile([NP, 1], F32, tag="s1sum")
                nc.vector.reduce_sum(out=s1sum, in_=accs[p], axis=X)
                r1 = srp.tile([NP, 1], F32, tag="r1")
                nc.vector.reciprocal(out=r1, in_=s1sum)
                avps = ps_small.tile([NP, 128], F32, tag="small")
                for l0 in range(NL):
                    nc.tensor.matmul(avps, lhsT=e1t[:, l0, :],
                                     rhs=vpbfs[l0][:, ts(p, 128)],
                                     start=(l0 == 0), stop=(l0 == NL - 1))
                avbd = avbd_pool.tile([NP, 128], BF16, tag="avbd")
                nc.vector.memset(avbd, 0.0)
                nc.vector.tensor_scalar_mul(out=avbd[0:A, 0:HD],
                                            in0=avps[0:A, 0:HD],
                                            scalar1=r1[0:A])
                nc.vector.tensor_scalar_mul(out=avbd[64:64 + A, HD:128],
                                            in0=avps[64:64 + A, HD:128],
                                            scalar1=r1[64:64 + A])
                avbds.append(avbd)

            # ============ PHASE D: stage-2 scores ============
            for p in range(PAIRS):
                e2 = e12_pool.tile([NP, L], BF16, tag="e12", name=f"exp2_{p}")
                for c in range(NCK):
                    nabt0 = bias_pool.tile([A, 512], BF16, tag="bias")
                    nc.gpsimd.dma_start(out=nabt0,
                                        in_=nab_d[2 * p, :, ts(c, 512)])
                    nabt1 = bias_pool.tile([A, 512], BF16, tag="bias")
                    nc.gpsimd.dma_start(out=nabt1,
                                        in_=nab_d[2 * p + 1, :, ts(c, 512)])
                    s2ps = ps_score.tile([NP, 512], F32, tag="score")
                    nc.tensor.matmul(s2ps, lhsT=bds[p],
                                     rhs=qT[:, p, ts(c, 512)],
                                     start=True, stop=False)
                    nc.tensor.matmul(s2ps, lhsT=iexta, rhs=nabt0,
                                     start=False, stop=False)
                    nc.tensor.matmul(s2ps, lhsT=iextb, rhs=nabt1,
                                     start=False, stop=True)
                    nc.scalar.activation(out=e2[:, ts(c, 512)],
                                         in_=s2ps, func=Exp)
                exp2s.append(e2)

            # ============ PHASE E: x = attn2 @ agent_v + lepe ============
            for l0 in range(NL):
                xps = ps_big.tile([128, CH], F32, tag="big")
                sums = ps_small.tile([128, 2 * PAIRS], F32, tag="small")
                for p in range(PAIRS):
                    nc.tensor.matmul(xps[:, ts(p, 128)],
                                     lhsT=exp2s[p][:, ts(l0, 128)],
                                     rhs=avbds[p], start=True, stop=True)
                    nc.tensor.matmul(sums[:, ts(p, 2)],
                                     lhsT=exp2s[p][:, ts(l0, 128)],
                                     rhs=ones2, start=True, stop=True)
                r2 = r2_pool.tile([128, 2 * PAIRS], F32, tag="r2")
                nc.vector.reciprocal(out=r2, in_=sums)
                nc.vector.tensor_mul(
                    out=xps.rearrange("p (g c) -> p g c", c=HD),
                    in0=xps.rearrange("p (g c) -> p g c", c=HD),
                    in1=r2.broadcast_to([128, 2 * PAIRS, HD]))
                nc.vector.tensor_add(out=vp32s[l0], in0=vp32s[l0], in1=xps)
                nc.sync.dma_start(out=out_d[ts(l0, 128), :], in_=vp32s[l0])

    nc.compile()
    return nc


def _host_consts():
    bf = ml_dtypes.bfloat16
    pt = _pool_matrix_T().astype(bf)
    iexta = np.zeros((A, NP), dtype=bf)
    iextb = np.zeros((A, NP), dtype=bf)
    for i in range(A):
        iexta[i, i] = 1.0
        iextb[i, 64 + i] = 1.0
    ones2 = np.zeros((NP, 2), dtype=bf)
    ones2[0:A, 0] = 1.0
    ones2[64:64 + A, 1] = 1.0
    id49 = np.eye(A, dtype=np.float32).astype(bf)
    return pt, iexta, iextb, ones2, id49


LAST_EXEC_NS = None
TRACE = False


def kernel(q, k, v, W_v, b_v, an_bias, na_bias):
    global LAST_EXEC_NS
    from concourse.bass_utils import run_bass_kernel_spmd

    if "nc" not in _CACHE:
        _CACHE["nc"] = _build_program()
    nc = _CACHE["nc"]

    q = np.asarray(q, np.float32)
    k = np.asarray(k, np.float32)
    v = np.asarray(v, np.float32)
    W_v = np.asarray(W_v, np.float32)
    b_v = np.asarray(b_v, np.float32)
    an_bias = np.asarray(an_bias, np.float32)
    na_bias = np.asarray(na_bias, np.float32)

    pt, iexta, iextb, ones2, id49 = _host_consts()
    in_maps = []
    for core in range(8):
        b, g = core // 2, core % 2
        cols = slice(CH * g, CH * (g + 1))
        heads = slice(8 * g, 8 * (g + 1))
        in_maps.append({
            "q_sh": np.ascontiguousarray(q[b][:, cols]),
            "k_sh": np.ascontiguousarray(k[b][:, cols]),
            "v_sh": np.ascontiguousarray(v[b]),
            "w_sh": np.ascontiguousarray(W_v[:, cols]),
            "bv_sh": np.ascontiguousarray(b_v[cols][None, :]),
            "anb_sh": np.ascontiguousarray(an_bias[heads]),
            "nab_sh": np.ascontiguousarray(na_bias[heads]),
            "pt_sh": pt, "iexta_sh": iexta, "iextb_sh": iextb,
            "ones2_sh": ones2, "id_sh": id49,
        })

    res = run_bass_kernel_spmd(nc, in_maps, list(range(8)), trace=TRACE)
    LAST_EXEC_NS = res.exec_time_ns

    out = np.empty((B, L, C), dtype=np.float32)
    for core in range(8):
        b, g = core // 2, core % 2
        out[b][:, CH * g:CH * (g + 1)] = res.results[core]["out_sh"]
    return out


if __name__ == "__main__":
    rng = np.random.default_rng(0)
    inp = {
        "q": rng.standard_normal((B, L, C)).astype(np.float32),
        "k": rng.standard_normal((B, L, C)).astype(np.float32),
        "v": rng.standard_normal((B, L, C)).astype(np.float32),
        "W_v": (rng.standard_normal((C, C)) * C ** -0.5).astype(np.float32),
        "b_v": (rng.standard_normal((C,)) * 0.02).astype(np.float32),
        "an_bias": (rng.standard_normal((H, A, L)) * 0.02).astype(np.float32),
        "na_bias": (rng.standard_normal((H, A, L)) * 0.02).astype(np.float32),
    }
    out = kernel(**inp)
    print(out.shape, out.dtype, LAST_EXEC_NS)
